# revision 50
# baseline (speedup 1.0000x reference)
"""CMSBlockLinear block-ELL sparse linear forward on 8 trn2 NeuronCores.

Strategy: the block-sparse weight (R=128 x K=32 active 16x16 tiles, 25%
density) is densified on the host into W^T [2048 in, 2048 out].  The device
runs a dense matmul y^T = W^T.T @ x^T with fp32 PSUM accumulation.
Dense-ifying costs 4x the weight FLOPs on paper, but the PE streams N
columns per matmul regardless of M, so a dense 128-wide M uses the array 8x
better than the natural M=16 sparse formulation.

Sharding (8 cores): 4-way over tokens x 2-way over output features.

Numeric config (error budget vs the 2e-2 gate; a host float64 sim of the
quantization matches HW rel_err to 7 digits, so this was tuned offline):
- Contraction chunks 0-3 ride in fp8(e4m3) as TWO DoubleRow pair passes
  (2 k-tiles per instruction, double-pumped PE): 4 chunks of progress for
  2 chunks of PE cycles and half the DMA bytes.
- Everything else (chunks 4-15, and the output y) is fp16 instead of bf16:
  same PE rate and DMA bytes, 8x less quantization error, which buys the
  margin for the second fp8 pair.  w8 is scaled x4 and x8 by 1/4 (exact
  powers of two, product invariant) to pull w's small values out of e4m3's
  subnormal range.  Measured rel_err 1.896706e-2, identical every run.

Device loop (trace-driven; v1 measured 44.4us, this version ~42.5 mean /
41.3 best with +-0.8us run-to-run jitter from the NEFF entry phase and
the volatile DMA completion-sem latency; the stream front is gated by the
second chunk's completion sem ~2us after its data lands, so warm-chain
length changes are zero-sum against it):
- The fp16 chunks OPEN the stream (k-outer m-inner, chunks 4..12), so the
  first real matmul gates on only xk4 + wk4's first half (~256KB); the
  fp8 pair passes run inside the m-major epilogue instead.  An fp8-first
  opener needs 768KB before pass B can finish and measurably drip-stalls.
  Chunks 4-6's weight DMAs are split into output-column halves: the
  split absorbed the measured k5 completion-sem stall and cut run-to-run
  sigma from ~0.9us to ~0.4us (8 runs, no draw above 43.1us).
- The fp8 DMAs are the LAST allocations of the x/w pools, so buffer-reuse
  pacing lands their 768KB late in the stream.  Do NOT move them earlier:
  any sizable DMA burst before ~17us wall time makes the DVFS governor
  park the PE at 2.0GHz (259ns/matmul instead of 216) for the WHOLE
  stream, measured +7us on 3/4 runs when released mid-stream.
- Warm-up: 8 dummy matmuls (last one half-width) hold the PE busy and
  ramp the DVFS clock until the opener's data lands (~body+4us: queue
  start body+1.7, transfer, plus a volatile 0.5-2us completion-semaphore
  latency).  PE idle mid-ramp delays the 2.4GHz grant by about the idle
  length.  N_WARM=7 triggered the 2.0GHz park on 3/3 runs (the schedule
  shift changes the early DMA picture) — treat the warm chain as
  load-bearing for the clock grant, not just as a bridge.
- Steady state is per-chunk demand-paced fp16 DMAs (x on sync HWDGE, w on
  scalar HWDGE, buffers rotating 5/6-deep): front-loading more keeps
  ~300GB/s in flight through the clock ramp and parks the clock (see
  above); deeper pools (7) also measured worse.  Ring headroom over the
  stream is only ~6%, so occasional 1-2us mid-stream stalls on bad
  completion-sem draws are expected and accepted.
- bias is applied on the host (zeros in this problem, exact in fp32
  either way).  Epilogue: last three fp16 chunks + both fp8 passes
  m-major so bank m closes ~1.1us before bank m+1; psum copies (even m
  on DVE, odd m on Scalar-ACT) + per-bank output pushes hide under the
  stream tail.  m7's copy runs as two halves on VECTOR (it wakes ~0.5us
  faster than scalar after the closing matmul) with the two 64KB pushes
  gen'd on sync/scalar in parallel right behind.
- ~7us NEFF entry (volatile +0-3.5us) and ~2.9us exit teardown are inside
  the measured window; the teardown does NOT scale with instruction/DMA
  count (v1 with 19 more matmuls had the same teardown), so don't chase
  semaphore-count reductions.
"""

import os

import numpy as np

BATCH, SEQ = 4, 512
IN_F = OUT_F = 2048
B = 16
R = 128  # output block rows
C = 128  # input block cols
KBLK = 32  # active tiles per row

TOK = BATCH * SEQ  # 2048 tokens
TOK_SHARDS = 4
OUT_SHARDS = 2
TOK_PER = TOK // TOK_SHARDS  # 512
OUT_PER = OUT_F // OUT_SHARDS  # 1024
K_CHUNKS = IN_F // 128  # 16
M_CHUNKS = OUT_PER // 128  # 8
N_FP8 = 4  # contraction chunks 0..3 in fp8 as two DoubleRow pairs
FP8_SCALE = 4.0  # w8 *= 4, x8 /= 4: rebalance e4m3 subnormal loss

# Warm slots bridge the first-DMA wait: the chain runs at the cold ~1.2GHz
# clock (~427ns per 512-wide matmul).  Measured body-relative: PE enters the
# body at ~body+0.65us and pass-A data is consumable at ~body+4.4us (queue
# start ~body+1.7, critical 256KB, ~1.9us completion-sem latency), so 8
# slots (~3.6us) bridge it.  PE idle mid-ramp delays the 2.4GHz grant by
# about the idle length, so the chain should end at (not before) arrival.
N_WARM = 8

LAST_EXEC_TIME_NS = None

_CACHE = {}


def _ensure_profile_hook():
    """Provide antenv.axon_hooks if the image lacks it, so trace=True works.

    Mirrors trn_agent_boot._ntff_profile_via_ctypes: drives NTFF capture via
    the libaxon_pjrt.so C ABI.  Also makes upload_artifacts fall back to the
    local dir when no artifact store is reachable.
    """
    import contextlib
    import ctypes
    import sys
    import types

    try:
        import antenv.axon_hooks  # noqa: F401

        return
    except ImportError:
        pass

    so_path = "/opt/axon/libaxon_pjrt.so"
    _hook = None
    if os.path.exists(so_path):
        try:
            lib = ctypes.CDLL(so_path)
            if hasattr(lib, "axon_start_nrt_profile"):
                lib.axon_start_nrt_profile.argtypes = [
                    ctypes.POINTER(ctypes.c_int64),
                    ctypes.c_size_t,
                ]
                lib.axon_start_nrt_profile.restype = ctypes.c_int64
                lib.axon_stop_nrt_profile.argtypes = [ctypes.c_char_p]
                lib.axon_stop_nrt_profile.restype = ctypes.c_int64

                @contextlib.contextmanager
                def _ntff_hook(output_dir, device_ids):
                    import jax

                    jax.devices()
                    if device_ids:
                        ids = (ctypes.c_int64 * len(device_ids))(*device_ids)
                        rc = lib.axon_start_nrt_profile(ids, len(device_ids))
                    else:
                        rc = lib.axon_start_nrt_profile(None, 0)
                    if rc != 0:
                        raise RuntimeError(f"axon_start_nrt_profile rc={rc}")
                    try:
                        yield
                    finally:
                        n = lib.axon_stop_nrt_profile(str(output_dir).encode())
                        print(f"profile: {n} file(s) -> {output_dir}", file=sys.stderr)

                _hook = _ntff_hook
        except OSError:
            pass

    mod = types.ModuleType("antenv.axon_hooks")
    mod.get_axon_ntff_profile_hook = lambda: _hook
    sys.modules["antenv.axon_hooks"] = mod

    import concourse.bass_utils as _bu

    _orig_upload = _bu.upload_artifacts

    def _safe_upload(tmpdir):
        try:
            return _orig_upload(tmpdir)
        except Exception:
            return tmpdir

    _bu.upload_artifacts = _safe_upload


def _build_nc():
    import concourse.mybir as mybir
    from concourse import bacc
    from concourse.tile import TileContext

    f16 = mybir.dt.float16
    fp8 = mybir.dt.float8e4

    nc = bacc.Bacc("TRN2", target_bir_lowering=False)
    xT = nc.dram_tensor("xT", [IN_F, TOK_PER], f16, kind="ExternalInput")
    w = nc.dram_tensor("w", [IN_F, OUT_PER], f16, kind="ExternalInput")
    # fp8 pieces.  Layout [p, t, :] with t = chunk index within the pair;
    # lhsT/rhs agree so the DoubleRow (p, t) reduction maps correctly.
    # Pair 1 (chunks 0-1) OPENS the stream: pre-2.4GHz-grant the PE runs at
    # 427ns/instr regardless, so a DoubleRow pair instr completes 432ns of
    # work there (~100% efficiency) where an fp16 matmul completes 216
    # (~50%).  Its weights split by output half so the opener gates on
    # x8a+w8p1L = 256KB only.  Pair 2 stays in the epilogue with its data
    # arriving late (same bytes as one fp16 chunk, zero front pressure).
    x8a = nc.dram_tensor("x8a", [128, 2, TOK_PER], fp8, kind="ExternalInput")
    x8b = nc.dram_tensor("x8b", [128, 2, TOK_PER], fp8, kind="ExternalInput")
    HO = OUT_PER // 2
    w8p1L = nc.dram_tensor("w8p1L", [128, 2, HO], fp8, kind="ExternalInput")
    w8p1R = nc.dram_tensor("w8p1R", [128, 2, HO], fp8, kind="ExternalInput")
    w8p2 = nc.dram_tensor("w8p2", [128, 2, OUT_PER], fp8, kind="ExternalInput")
    # y device layout: [partition, col-group, token] with col-groups
    # [m0,m2,m4,m6,m1,m3,m5,m7] - 1-2 KB contiguous per (partition, push).
    # Host un-permutes.
    y = nc.dram_tensor("y", [128, M_CHUNKS * TOK_PER], f16, kind="ExternalOutput")

    with TileContext(nc) as tc:
        with (
            tc.tile_pool(name="consts", bufs=1) as consts,
            tc.tile_pool(name="xp", bufs=5) as xp,
            tc.tile_pool(name="wp", bufs=6) as wp,
            tc.tile_pool(name="late", bufs=1) as late,
            tc.tile_pool(name="op", bufs=1) as op,
            tc.tile_pool(name="ps", bufs=1, space="PSUM") as ps,
        ):
            psums = [
                ps.tile([128, TOK_PER], mybir.dt.float32, tag=f"ps{m}", name=f"ps{m}")
                for m in range(M_CHUNKS)
            ]

            # Warm-up: dummy matmuls hold the PE busy (and ramp the DVFS
            # clock) until pass A's data lands.  Contents irrelevant (pass
            # A's start=True resets each bank), but Tile needs a writer to
            # allocate the tile - one cheap column memset suffices.
            warm = consts.tile([128, TOK_PER], f16)
            nc.vector.memset(warm[:, :1], 0)
            for i in range(N_WARM):
                # The last slot is half-width: finer bridge granularity so
                # the chain ends closer to (not past) the data-ready point.
                wn = TOK_PER // 2 if i == N_WARM - 1 else TOK_PER
                nc.tensor.matmul(
                    psums[0][:, :wn],
                    warm[:, :128],
                    warm[:, :wn],
                    start=(i == 0),
                    stop=(i == N_WARM - 1),
                )

            # Input DMAs.  Pair 1's pieces go FIRST on the rings (opener
            # gates on x8a+w8p1L = 256KB), then the fp16 chunks with
            # 5/6-deep demand pacing — that pacing protects the DVFS
            # governor's early window (any sizable DMA burst before ~wall
            # 17us parks the PE at 2.0GHz for the whole stream, +7us; a
            # batched late release at ~18us also measured ~1.5us worse).
            # Pair 2's pieces are singleton-tag tiles (no reuse, no pacing
            # waits) emitted last: their descriptor gens queue behind the
            # last paced chunk gens, landing their 384KB late in the
            # stream where its pass runs (the epilogue).
            x8at = late.tile([128, 2, TOK_PER], fp8, name="x8at", tag="x8at")
            w8p1Lt = late.tile([128, 2, HO], fp8, name="w8p1Lt", tag="w8p1Lt")
            w8p1Rt = late.tile([128, 2, HO], fp8, name="w8p1Rt", tag="w8p1Rt")
            nc.sync.dma_start(x8at[:], x8a[:])
            nc.scalar.dma_start(w8p1Lt[:], w8p1L[:])
            # NOTE: splitting w8p1R into column halves (to shave pair-1's
            # occasional m4 sem stall) drew 45us+ parks on 2/4 runs —
            # keep it whole.
            nc.scalar.dma_start(w8p1Rt[:], w8p1R[:])
            xks, wks = [], []
            for k in range(K_CHUNKS):
                if k < N_FP8:
                    xks.append(None)
                    wks.append(None)
                    continue
                xk = xp.tile([128, TOK_PER], f16, name=f"xk{k}", tag="xk")
                wk = wp.tile([128, OUT_PER], f16, name=f"wk{k}", tag="wk")
                nc.sync.dma_start(xk[:], xT[k * 128 : (k + 1) * 128, :])
                if k <= N_FP8 + 2:
                    # The first three chunks' weights in two halves: chunk
                    # k5's m0-3 matmuls gate the whole stream on wk5's
                    # completion sem (measured ~13.6us, ~2us after the data
                    # lands), so halving the piece that sem covers pulls the
                    # gate ~0.5us earlier; k4/k6 split for the same reason
                    # on bad draws.
                    nc.scalar.dma_start(
                        wk[:, 0 : OUT_PER // 2],
                        w[k * 128 : (k + 1) * 128, 0 : OUT_PER // 2],
                    )
                    nc.scalar.dma_start(
                        wk[:, OUT_PER // 2 : OUT_PER],
                        w[k * 128 : (k + 1) * 128, OUT_PER // 2 : OUT_PER],
                    )
                else:
                    nc.scalar.dma_start(wk[:], w[k * 128 : (k + 1) * 128, :])
                xks.append(xk)
                wks.append(wk)
            x8bt = late.tile([128, 2, TOK_PER], fp8, name="x8bt", tag="x8bt")
            w8p2t = late.tile([128, 2, OUT_PER], fp8, name="w8p2t", tag="w8p2t")
            nc.sync.dma_start(x8bt[:], x8b[:])
            nc.scalar.dma_start(w8p2t[:], w8p2[:])

            # Pair-1 opener: 8 DoubleRow instrs, each 2 chunks of progress,
            # running mostly in the pre-grant slow-clock window where they
            # are ~2x as work-dense as fp16 matmuls.  start=True clears the
            # banks after the warm junk.
            for m in range(M_CHUNKS):
                w8t = w8p1Lt if m < 4 else w8p1Rt
                mm = m % 4
                nc.tensor.matmul(
                    psums[m][:],
                    w8t[:, :, mm * 128 : (mm + 1) * 128],
                    x8at[:],
                    start=True,
                    stop=False,
                    perf_mode=mybir.MatmulPerfMode.DoubleRow,
                )

            # Steady state: k-outer, m-inner (fp16 chunks 4..12).
            for k in range(N_FP8, K_CHUNKS - 3):
                for m in range(M_CHUNKS):
                    nc.tensor.matmul(
                        psums[m][:],
                        wks[k][:, m * 128 : (m + 1) * 128],
                        xks[k][:],
                        start=False,
                        stop=False,
                    )

            outA = op.tile([128, M_CHUNKS // 2, TOK_PER], f16, name="outA")
            outB = op.tile([128, M_CHUNKS // 2, TOK_PER], f16, name="outB")

            # Epilogue: last three fp16 chunks PLUS the pair-2 DoubleRow
            # pass, m-major, so bank m closes ~0.86us before bank m+1;
            # each bank's copy and each output DMA push is emitted right
            # behind its close and overlaps the stream tail.
            T = TOK_PER
            H = TOK_PER // 2
            for m in range(M_CHUNKS):
                j = m // 2
                if m == M_CHUNKS - 1:
                    # NOTE: a token-half split of this bank's epilogue (copy
                    # h0 while h1's matmuls run) measured NET-NEGATIVE: Tile
                    # serializes the h1 matmuls behind the h0 copy (PSUM WAR
                    # hazard at bank granularity), inserting a ~550ns PE
                    # stall that exceeds the tail saving.  Full-width close,
                    # then both half-copies on VECTOR (wakes fast; scalar
                    # wakes ~0.5us late; gpsimd cannot read PSUM — walrus
                    # rejects it).  The push gens (sync/scalar) start
                    # concurrently with the copies — the queue waits on the
                    # copy sem, not the gen.
                    for kk in range(K_CHUNKS - 3, K_CHUNKS):
                        nc.tensor.matmul(
                            psums[m][:],
                            wks[kk][:, m * 128 : (m + 1) * 128],
                            xks[kk][:],
                            start=False,
                            stop=False,
                        )
                    nc.tensor.matmul(
                        psums[m][:],
                        w8p2t[:, :, m * 128 : (m + 1) * 128],
                        x8bt[:],
                        start=False,
                        stop=True,
                        perf_mode=mybir.MatmulPerfMode.DoubleRow,
                    )
                    nc.vector.tensor_scalar_add(outB[:, j, 0:H], psums[m][:, 0:H], 0.0)
                    nc.vector.tensor_scalar_add(outB[:, j, H:T], psums[m][:, H:T], 0.0)
                    nc.sync.dma_start(y[:, 7 * T : 7 * T + H], outB[:, j, 0:H])
                    nc.scalar.dma_start(y[:, 7 * T + H : 8 * T], outB[:, j, H:T])
                    continue
                for kk in range(K_CHUNKS - 3, K_CHUNKS):
                    nc.tensor.matmul(
                        psums[m][:],
                        wks[kk][:, m * 128 : (m + 1) * 128],
                        xks[kk][:],
                        start=False,
                        stop=False,
                    )
                nc.tensor.matmul(
                    psums[m][:],
                    w8p2t[:, :, m * 128 : (m + 1) * 128],
                    x8bt[:],
                    start=False,
                    stop=True,
                    perf_mode=mybir.MatmulPerfMode.DoubleRow,
                )
                if m % 2 == 0:
                    nc.vector.tensor_scalar_add(outA[:, j, :], psums[m][:], 0.0)
                else:
                    nc.scalar.copy(outB[:, j, :], psums[m][:])

                if m == 2:
                    nc.sync.dma_start(y[:, 0 : 2 * T], outA[:, 0:2, :])  # m0,m2
                elif m == 3:
                    nc.scalar.dma_start(y[:, 4 * T : 6 * T], outB[:, 0:2, :])  # m1,m3
                elif m == 4:
                    nc.sync.dma_start(y[:, 2 * T : 3 * T], outA[:, 2:3, :])  # m4
                elif m == 5:
                    nc.sync.dma_start(y[:, 6 * T : 7 * T], outB[:, 2:3, :])  # m5
                elif m == 6:
                    # m6 pushed alone (m4 already went) so the final drain
                    # backlog is only m6+m7.
                    nc.sync.dma_start(y[:, 3 * T : 4 * T], outA[:, 3:4, :])  # m6

    nc.finalize()
    return nc


def _densify_wT(values: np.ndarray, col_indices: np.ndarray) -> np.ndarray:
    """W^T [in=2048, out=2048] with W[r*16+i, c*16+j] = values[r,k,i,j]."""
    wT = np.zeros((C, B, R, B), dtype=np.float32)  # [c, j, r, i]
    vals_t = values.transpose(0, 1, 3, 2)  # [R, K, j, i]
    r_idx = np.arange(R)
    wT[col_indices, :, r_idx[:, None], :] = vals_t
    return wT.reshape(IN_F, OUT_F)


def kernel(x, values, col_indices, bias):
    global LAST_EXEC_TIME_NS
    import ml_dtypes

    _ensure_profile_hook()
    from concourse.bass_utils import run_bass_kernel_spmd

    if "nc" not in _CACHE:
        _CACHE["nc"] = _build_nc()
    nc = _CACHE["nc"]

    f16 = np.float16
    fp8 = ml_dtypes.float8_e4m3
    wT32 = _densify_wT(np.asarray(values), np.asarray(col_indices))
    xT32 = np.ascontiguousarray(np.asarray(x, dtype=np.float32).reshape(TOK, IN_F).T)
    wT = wT32.astype(f16)
    xT = xT32.astype(f16)
    bias_f = np.asarray(bias, dtype=np.float32)

    def _pack8(src32, n_t, width):
        # [n_t*128 rows, width] fp32 -> [128, n_t, width] fp8 with
        # [p, t, :] = row t*128+p (must match the device (p, t) map).
        return np.ascontiguousarray(
            src32.astype(fp8).reshape(n_t, 128, width).transpose(1, 0, 2)
        )

    in_maps = []
    for core in range(8):
        t, h = divmod(core, OUT_SHARDS)
        xs = slice(t * TOK_PER, (t + 1) * TOK_PER)
        ws = slice(h * OUT_PER, (h + 1) * OUT_PER)
        x8_src = xT32[: N_FP8 * 128, xs] * (1.0 / FP8_SCALE)
        w8_src = wT32[: N_FP8 * 128, ws] * FP8_SCALE
        in_maps.append(
            {
                "xT": np.ascontiguousarray(xT[:, xs]),
                "w": np.ascontiguousarray(wT[:, ws]),
                "x8a": _pack8(x8_src[0:256], 2, TOK_PER),
                "x8b": _pack8(x8_src[256:512], 2, TOK_PER),
                "w8p1L": _pack8(w8_src[0:256, 0 : OUT_PER // 2], 2, OUT_PER // 2),
                "w8p1R": _pack8(
                    w8_src[0:256, OUT_PER // 2 : OUT_PER], 2, OUT_PER // 2
                ),
                "w8p2": _pack8(w8_src[256:512, :], 2, OUT_PER),
            }
        )

    res = run_bass_kernel_spmd(
        nc,
        in_maps,
        list(range(8)),
        trace=bool(os.environ.get("BASS_TRACE")),
    )
    LAST_EXEC_TIME_NS = res.exec_time_ns

    y = np.empty((TOK, OUT_F), dtype=np.float32)
    for core in range(8):
        t, h = divmod(core, OUT_SHARDS)
        # [128, 8, TOK_PER] with col-groups g -> m = [0,2,4,6,1,3,5,7][g]
        y_dev = (
            res.results[core]["y"]
            .astype(np.float32)
            .reshape(128, M_CHUNKS, TOK_PER)
            .transpose(1, 0, 2)  # [g, p, t]
        )
        y_log = y_dev[[0, 4, 1, 5, 2, 6, 3, 7]].reshape(OUT_PER, TOK_PER)
        y[t * TOK_PER : (t + 1) * TOK_PER, h * OUT_PER : (h + 1) * OUT_PER] = y_log.T
    return (y + bias_f[None, :]).reshape(BATCH, SEQ, OUT_F)


# revision 51
# speedup vs baseline: 1.0578x; 1.0578x over previous
"""CMSBlockLinear block-ELL sparse linear forward on 8 trn2 NeuronCores.

Strategy: the block-sparse weight (R=128 x K=32 active 16x16 tiles, 25%
density) is densified on the host into W^T [2048 in, 2048 out].  The device
runs a dense matmul y^T = W^T.T @ x^T with fp32 PSUM accumulation.
Dense-ifying costs 4x the weight FLOPs on paper, but the PE streams N
columns per matmul regardless of M, so a dense 128-wide M uses the array 8x
better than the natural M=16 sparse formulation.

Sharding (8 cores): 4-way over tokens x 2-way over output features.

Numeric config (error budget vs the 2e-2 gate; a host float64 sim of the
quantization matches HW rel_err to 7 digits, so this was tuned offline):
- Contraction chunks 0-3 ride in fp8(e4m3) as TWO DoubleRow pair passes
  (2 k-tiles per instruction, double-pumped PE): 4 chunks of progress for
  2 chunks of PE cycles and half the DMA bytes.
- Everything else (chunks 4-15, and the output y) is fp16 instead of bf16:
  same PE rate and DMA bytes, 8x less quantization error, which buys the
  margin for the second fp8 pair.  w8 is scaled x4 and x8 by 1/4 (exact
  powers of two, product invariant) to pull w's small values out of e4m3's
  subnormal range.  Measured rel_err 1.896706e-2, identical every run.

Device loop (trace-driven; v1 measured 44.4us, this version ~42.5 mean /
41.3 best with +-0.8us run-to-run jitter from the NEFF entry phase and
the volatile DMA completion-sem latency; the stream front is gated by the
second chunk's completion sem ~2us after its data lands, so warm-chain
length changes are zero-sum against it):
- The fp16 chunks OPEN the stream (k-outer m-inner, chunks 4..12), so the
  first real matmul gates on only xk4 + wk4's first half (~256KB); the
  fp8 pair passes run inside the m-major epilogue instead.  An fp8-first
  opener needs 768KB before pass B can finish and measurably drip-stalls.
  Chunks 4-6's weight DMAs are split into output-column halves: the
  split absorbed the measured k5 completion-sem stall and cut run-to-run
  sigma from ~0.9us to ~0.4us (8 runs, no draw above 43.1us).
- The fp8 DMAs are the LAST allocations of the x/w pools, so buffer-reuse
  pacing lands their 768KB late in the stream.  Do NOT move them earlier:
  any sizable DMA burst before ~17us wall time makes the DVFS governor
  park the PE at 2.0GHz (259ns/matmul instead of 216) for the WHOLE
  stream, measured +7us on 3/4 runs when released mid-stream.
- Warm-up: 8 dummy matmuls (last one half-width) hold the PE busy and
  ramp the DVFS clock until the opener's data lands (~body+4us: queue
  start body+1.7, transfer, plus a volatile 0.5-2us completion-semaphore
  latency).  PE idle mid-ramp delays the 2.4GHz grant by about the idle
  length.  N_WARM=7 triggered the 2.0GHz park on 3/3 runs (the schedule
  shift changes the early DMA picture) — treat the warm chain as
  load-bearing for the clock grant, not just as a bridge.
- Steady state is per-chunk demand-paced fp16 DMAs (x on sync HWDGE, w on
  scalar HWDGE, buffers rotating 5/6-deep): front-loading more keeps
  ~300GB/s in flight through the clock ramp and parks the clock (see
  above); deeper pools (7) also measured worse.  Ring headroom over the
  stream is only ~6%, so occasional 1-2us mid-stream stalls on bad
  completion-sem draws are expected and accepted.
- bias is applied on the host (zeros in this problem, exact in fp32
  either way).  Epilogue: last three fp16 chunks + both fp8 passes
  m-major so bank m closes ~1.1us before bank m+1; psum copies (even m
  on DVE, odd m on Scalar-ACT) + per-bank output pushes hide under the
  stream tail.  m7's copy runs as two halves on VECTOR (it wakes ~0.5us
  faster than scalar after the closing matmul) with the two 64KB pushes
  gen'd on sync/scalar in parallel right behind.
- ~7us NEFF entry (volatile +0-3.5us) and ~2.9us exit teardown are inside
  the measured window; the teardown does NOT scale with instruction/DMA
  count (v1 with 19 more matmuls had the same teardown), so don't chase
  semaphore-count reductions.
"""

import os

import numpy as np

BATCH, SEQ = 4, 512
IN_F = OUT_F = 2048
B = 16
R = 128  # output block rows
C = 128  # input block cols
KBLK = 32  # active tiles per row

TOK = BATCH * SEQ  # 2048 tokens
TOK_SHARDS = 4
OUT_SHARDS = 2
TOK_PER = TOK // TOK_SHARDS  # 512
OUT_PER = OUT_F // OUT_SHARDS  # 1024
K_CHUNKS = IN_F // 128  # 16
M_CHUNKS = OUT_PER // 128  # 8
N_FP8 = 4  # contraction chunks 0..3 in fp8 as two DoubleRow pairs
FP8_SCALE = 4.0  # w8 *= 4, x8 /= 4: rebalance e4m3 subnormal loss

# Warm slots bridge the first-DMA wait: the chain runs at the cold ~1.2GHz
# clock (~427ns per 512-wide matmul).  Measured body-relative: PE enters the
# body at ~body+0.65us and pass-A data is consumable at ~body+4.4us (queue
# start ~body+1.7, critical 256KB, ~1.9us completion-sem latency), so 8
# slots (~3.6us) bridge it.  PE idle mid-ramp delays the 2.4GHz grant by
# about the idle length, so the chain should end at (not before) arrival.
N_WARM = 7

LAST_EXEC_TIME_NS = None

_CACHE = {}


def _ensure_profile_hook():
    """Provide antenv.axon_hooks if the image lacks it, so trace=True works.

    Mirrors trn_agent_boot._ntff_profile_via_ctypes: drives NTFF capture via
    the libaxon_pjrt.so C ABI.  Also makes upload_artifacts fall back to the
    local dir when no artifact store is reachable.
    """
    import contextlib
    import ctypes
    import sys
    import types

    try:
        import antenv.axon_hooks  # noqa: F401

        return
    except ImportError:
        pass

    so_path = "/opt/axon/libaxon_pjrt.so"
    _hook = None
    if os.path.exists(so_path):
        try:
            lib = ctypes.CDLL(so_path)
            if hasattr(lib, "axon_start_nrt_profile"):
                lib.axon_start_nrt_profile.argtypes = [
                    ctypes.POINTER(ctypes.c_int64),
                    ctypes.c_size_t,
                ]
                lib.axon_start_nrt_profile.restype = ctypes.c_int64
                lib.axon_stop_nrt_profile.argtypes = [ctypes.c_char_p]
                lib.axon_stop_nrt_profile.restype = ctypes.c_int64

                @contextlib.contextmanager
                def _ntff_hook(output_dir, device_ids):
                    import jax

                    jax.devices()
                    if device_ids:
                        ids = (ctypes.c_int64 * len(device_ids))(*device_ids)
                        rc = lib.axon_start_nrt_profile(ids, len(device_ids))
                    else:
                        rc = lib.axon_start_nrt_profile(None, 0)
                    if rc != 0:
                        raise RuntimeError(f"axon_start_nrt_profile rc={rc}")
                    try:
                        yield
                    finally:
                        n = lib.axon_stop_nrt_profile(str(output_dir).encode())
                        print(f"profile: {n} file(s) -> {output_dir}", file=sys.stderr)

                _hook = _ntff_hook
        except OSError:
            pass

    mod = types.ModuleType("antenv.axon_hooks")
    mod.get_axon_ntff_profile_hook = lambda: _hook
    sys.modules["antenv.axon_hooks"] = mod

    import concourse.bass_utils as _bu

    _orig_upload = _bu.upload_artifacts

    def _safe_upload(tmpdir):
        try:
            return _orig_upload(tmpdir)
        except Exception:
            return tmpdir

    _bu.upload_artifacts = _safe_upload


def _build_nc():
    import concourse.mybir as mybir
    from concourse import bacc
    from concourse.tile import TileContext

    f16 = mybir.dt.float16
    fp8 = mybir.dt.float8e4

    nc = bacc.Bacc("TRN2", target_bir_lowering=False)
    xT = nc.dram_tensor("xT", [IN_F, TOK_PER], f16, kind="ExternalInput")
    w = nc.dram_tensor("w", [IN_F, OUT_PER], f16, kind="ExternalInput")
    # fp8 pieces.  Layout [p, t, :] with t = chunk index within the pair;
    # lhsT/rhs agree so the DoubleRow (p, t) reduction maps correctly.
    # Pair 1 (chunks 0-1) OPENS the stream: pre-2.4GHz-grant the PE runs at
    # 427ns/instr regardless, so a DoubleRow pair instr completes 432ns of
    # work there (~100% efficiency) where an fp16 matmul completes 216
    # (~50%).  Its weights split by output half so the opener gates on
    # x8a+w8p1L = 256KB only.  Pair 2 stays in the epilogue with its data
    # arriving late (same bytes as one fp16 chunk, zero front pressure).
    x8a = nc.dram_tensor("x8a", [128, 2, TOK_PER], fp8, kind="ExternalInput")
    x8b = nc.dram_tensor("x8b", [128, 2, TOK_PER], fp8, kind="ExternalInput")
    HO = OUT_PER // 2
    w8p1L = nc.dram_tensor("w8p1L", [128, 2, HO], fp8, kind="ExternalInput")
    w8p1R = nc.dram_tensor("w8p1R", [128, 2, HO], fp8, kind="ExternalInput")
    w8p2 = nc.dram_tensor("w8p2", [128, 2, OUT_PER], fp8, kind="ExternalInput")
    # y device layout: [partition, col-group, token] with col-groups
    # [m0,m2,m4,m6,m1,m3,m5,m7] - 1-2 KB contiguous per (partition, push).
    # Host un-permutes.
    y = nc.dram_tensor("y", [128, M_CHUNKS * TOK_PER], f16, kind="ExternalOutput")

    with TileContext(nc) as tc:
        with (
            tc.tile_pool(name="consts", bufs=1) as consts,
            tc.tile_pool(name="xp", bufs=5) as xp,
            tc.tile_pool(name="wp", bufs=6) as wp,
            tc.tile_pool(name="late", bufs=1) as late,
            tc.tile_pool(name="op", bufs=1) as op,
            tc.tile_pool(name="ps", bufs=1, space="PSUM") as ps,
        ):
            psums = [
                ps.tile([128, TOK_PER], mybir.dt.float32, tag=f"ps{m}", name=f"ps{m}")
                for m in range(M_CHUNKS)
            ]

            # Warm-up: dummy matmuls hold the PE busy (and ramp the DVFS
            # clock) until pass A's data lands.  Contents irrelevant (pass
            # A's start=True resets each bank), but Tile needs a writer to
            # allocate the tile - one cheap column memset suffices.
            warm = consts.tile([128, TOK_PER], f16)
            nc.vector.memset(warm[:, :1], 0)
            for i in range(N_WARM):
                # The last slot is half-width: finer bridge granularity so
                # the chain ends closer to (not past) the data-ready point.
                wn = TOK_PER // 2 if i == N_WARM - 1 else TOK_PER
                nc.tensor.matmul(
                    psums[0][:, :wn],
                    warm[:, :128],
                    warm[:, :wn],
                    start=(i == 0),
                    stop=(i == N_WARM - 1),
                )

            # Input DMAs.  Pair 1's pieces go FIRST on the rings (opener
            # gates on x8a+w8p1L = 256KB), then the fp16 chunks with
            # 5/6-deep demand pacing — that pacing protects the DVFS
            # governor's early window (any sizable DMA burst before ~wall
            # 17us parks the PE at 2.0GHz for the whole stream, +7us; a
            # batched late release at ~18us also measured ~1.5us worse).
            # Pair 2's pieces are singleton-tag tiles (no reuse, no pacing
            # waits) emitted last: their descriptor gens queue behind the
            # last paced chunk gens, landing their 384KB late in the
            # stream where its pass runs (the epilogue).
            x8at = late.tile([128, 2, TOK_PER], fp8, name="x8at", tag="x8at")
            w8p1Lt = late.tile([128, 2, HO], fp8, name="w8p1Lt", tag="w8p1Lt")
            w8p1Rt = late.tile([128, 2, HO], fp8, name="w8p1Rt", tag="w8p1Rt")
            nc.sync.dma_start(x8at[:], x8a[:])
            nc.scalar.dma_start(w8p1Lt[:], w8p1L[:])
            # NOTE: splitting w8p1R into column halves (to shave pair-1's
            # occasional m4 sem stall) drew 45us+ parks on 2/4 runs —
            # keep it whole.
            nc.scalar.dma_start(w8p1Rt[:], w8p1R[:])
            xks, wks = [], []
            for k in range(K_CHUNKS):
                if k < N_FP8:
                    xks.append(None)
                    wks.append(None)
                    continue
                xk = xp.tile([128, TOK_PER], f16, name=f"xk{k}", tag="xk")
                wk = wp.tile([128, OUT_PER], f16, name=f"wk{k}", tag="wk")
                nc.sync.dma_start(xk[:], xT[k * 128 : (k + 1) * 128, :])
                if k <= N_FP8 + 2:
                    # The first three chunks' weights in two halves: chunk
                    # k5's m0-3 matmuls gate the whole stream on wk5's
                    # completion sem (measured ~13.6us, ~2us after the data
                    # lands), so halving the piece that sem covers pulls the
                    # gate ~0.5us earlier; k4/k6 split for the same reason
                    # on bad draws.
                    nc.scalar.dma_start(
                        wk[:, 0 : OUT_PER // 2],
                        w[k * 128 : (k + 1) * 128, 0 : OUT_PER // 2],
                    )
                    nc.scalar.dma_start(
                        wk[:, OUT_PER // 2 : OUT_PER],
                        w[k * 128 : (k + 1) * 128, OUT_PER // 2 : OUT_PER],
                    )
                else:
                    nc.scalar.dma_start(wk[:], w[k * 128 : (k + 1) * 128, :])
                xks.append(xk)
                wks.append(wk)
            x8bt = late.tile([128, 2, TOK_PER], fp8, name="x8bt", tag="x8bt")
            w8p2t = late.tile([128, 2, OUT_PER], fp8, name="w8p2t", tag="w8p2t")
            nc.sync.dma_start(x8bt[:], x8b[:])
            nc.scalar.dma_start(w8p2t[:], w8p2[:])

            # Pair-1 opener: 8 DoubleRow instrs, each 2 chunks of progress,
            # running mostly in the pre-grant slow-clock window where they
            # are ~2x as work-dense as fp16 matmuls.  start=True clears the
            # banks after the warm junk.
            for m in range(M_CHUNKS):
                w8t = w8p1Lt if m < 4 else w8p1Rt
                mm = m % 4
                nc.tensor.matmul(
                    psums[m][:],
                    w8t[:, :, mm * 128 : (mm + 1) * 128],
                    x8at[:],
                    start=True,
                    stop=False,
                    perf_mode=mybir.MatmulPerfMode.DoubleRow,
                )

            # Steady state: k-outer, m-inner (fp16 chunks 4..12).
            for k in range(N_FP8, K_CHUNKS - 3):
                for m in range(M_CHUNKS):
                    nc.tensor.matmul(
                        psums[m][:],
                        wks[k][:, m * 128 : (m + 1) * 128],
                        xks[k][:],
                        start=False,
                        stop=False,
                    )

            outA = op.tile([128, M_CHUNKS // 2, TOK_PER], f16, name="outA")
            outB = op.tile([128, M_CHUNKS // 2, TOK_PER], f16, name="outB")

            # Epilogue: last three fp16 chunks PLUS the pair-2 DoubleRow
            # pass, m-major, so bank m closes ~0.86us before bank m+1;
            # each bank's copy and each output DMA push is emitted right
            # behind its close and overlaps the stream tail.
            T = TOK_PER
            H = TOK_PER // 2
            for m in range(M_CHUNKS):
                j = m // 2
                if m == M_CHUNKS - 1:
                    # NOTE: a token-half split of this bank's epilogue (copy
                    # h0 while h1's matmuls run) measured NET-NEGATIVE: Tile
                    # serializes the h1 matmuls behind the h0 copy (PSUM WAR
                    # hazard at bank granularity), inserting a ~550ns PE
                    # stall that exceeds the tail saving.  Full-width close,
                    # then both half-copies on VECTOR (wakes fast; scalar
                    # wakes ~0.5us late; gpsimd cannot read PSUM — walrus
                    # rejects it).  The push gens (sync/scalar) start
                    # concurrently with the copies — the queue waits on the
                    # copy sem, not the gen.
                    for kk in range(K_CHUNKS - 3, K_CHUNKS):
                        nc.tensor.matmul(
                            psums[m][:],
                            wks[kk][:, m * 128 : (m + 1) * 128],
                            xks[kk][:],
                            start=False,
                            stop=False,
                        )
                    nc.tensor.matmul(
                        psums[m][:],
                        w8p2t[:, :, m * 128 : (m + 1) * 128],
                        x8bt[:],
                        start=False,
                        stop=True,
                        perf_mode=mybir.MatmulPerfMode.DoubleRow,
                    )
                    nc.vector.tensor_scalar_add(outB[:, j, 0:H], psums[m][:, 0:H], 0.0)
                    nc.vector.tensor_scalar_add(outB[:, j, H:T], psums[m][:, H:T], 0.0)
                    nc.sync.dma_start(y[:, 7 * T : 7 * T + H], outB[:, j, 0:H])
                    nc.scalar.dma_start(y[:, 7 * T + H : 8 * T], outB[:, j, H:T])
                    continue
                for kk in range(K_CHUNKS - 3, K_CHUNKS):
                    nc.tensor.matmul(
                        psums[m][:],
                        wks[kk][:, m * 128 : (m + 1) * 128],
                        xks[kk][:],
                        start=False,
                        stop=False,
                    )
                nc.tensor.matmul(
                    psums[m][:],
                    w8p2t[:, :, m * 128 : (m + 1) * 128],
                    x8bt[:],
                    start=False,
                    stop=True,
                    perf_mode=mybir.MatmulPerfMode.DoubleRow,
                )
                if m % 2 == 0:
                    nc.vector.tensor_scalar_add(outA[:, j, :], psums[m][:], 0.0)
                else:
                    nc.scalar.copy(outB[:, j, :], psums[m][:])

                if m == 2:
                    nc.sync.dma_start(y[:, 0 : 2 * T], outA[:, 0:2, :])  # m0,m2
                elif m == 3:
                    nc.scalar.dma_start(y[:, 4 * T : 6 * T], outB[:, 0:2, :])  # m1,m3
                elif m == 4:
                    nc.sync.dma_start(y[:, 2 * T : 3 * T], outA[:, 2:3, :])  # m4
                elif m == 5:
                    nc.sync.dma_start(y[:, 6 * T : 7 * T], outB[:, 2:3, :])  # m5
                elif m == 6:
                    # m6 pushed alone (m4 already went) so the final drain
                    # backlog is only m6+m7.
                    nc.sync.dma_start(y[:, 3 * T : 4 * T], outA[:, 3:4, :])  # m6

    nc.finalize()
    return nc


def _densify_wT(values: np.ndarray, col_indices: np.ndarray) -> np.ndarray:
    """W^T [in=2048, out=2048] with W[r*16+i, c*16+j] = values[r,k,i,j]."""
    wT = np.zeros((C, B, R, B), dtype=np.float32)  # [c, j, r, i]
    vals_t = values.transpose(0, 1, 3, 2)  # [R, K, j, i]
    r_idx = np.arange(R)
    wT[col_indices, :, r_idx[:, None], :] = vals_t
    return wT.reshape(IN_F, OUT_F)


def kernel(x, values, col_indices, bias):
    global LAST_EXEC_TIME_NS
    import ml_dtypes

    _ensure_profile_hook()
    from concourse.bass_utils import run_bass_kernel_spmd

    if "nc" not in _CACHE:
        _CACHE["nc"] = _build_nc()
    nc = _CACHE["nc"]

    f16 = np.float16
    fp8 = ml_dtypes.float8_e4m3
    wT32 = _densify_wT(np.asarray(values), np.asarray(col_indices))
    xT32 = np.ascontiguousarray(np.asarray(x, dtype=np.float32).reshape(TOK, IN_F).T)
    wT = wT32.astype(f16)
    xT = xT32.astype(f16)
    bias_f = np.asarray(bias, dtype=np.float32)

    def _pack8(src32, n_t, width):
        # [n_t*128 rows, width] fp32 -> [128, n_t, width] fp8 with
        # [p, t, :] = row t*128+p (must match the device (p, t) map).
        return np.ascontiguousarray(
            src32.astype(fp8).reshape(n_t, 128, width).transpose(1, 0, 2)
        )

    in_maps = []
    for core in range(8):
        t, h = divmod(core, OUT_SHARDS)
        xs = slice(t * TOK_PER, (t + 1) * TOK_PER)
        ws = slice(h * OUT_PER, (h + 1) * OUT_PER)
        x8_src = xT32[: N_FP8 * 128, xs] * (1.0 / FP8_SCALE)
        w8_src = wT32[: N_FP8 * 128, ws] * FP8_SCALE
        in_maps.append(
            {
                "xT": np.ascontiguousarray(xT[:, xs]),
                "w": np.ascontiguousarray(wT[:, ws]),
                "x8a": _pack8(x8_src[0:256], 2, TOK_PER),
                "x8b": _pack8(x8_src[256:512], 2, TOK_PER),
                "w8p1L": _pack8(w8_src[0:256, 0 : OUT_PER // 2], 2, OUT_PER // 2),
                "w8p1R": _pack8(
                    w8_src[0:256, OUT_PER // 2 : OUT_PER], 2, OUT_PER // 2
                ),
                "w8p2": _pack8(w8_src[256:512, :], 2, OUT_PER),
            }
        )

    res = run_bass_kernel_spmd(
        nc,
        in_maps,
        list(range(8)),
        trace=bool(os.environ.get("BASS_TRACE")),
    )
    LAST_EXEC_TIME_NS = res.exec_time_ns

    y = np.empty((TOK, OUT_F), dtype=np.float32)
    for core in range(8):
        t, h = divmod(core, OUT_SHARDS)
        # [128, 8, TOK_PER] with col-groups g -> m = [0,2,4,6,1,3,5,7][g]
        y_dev = (
            res.results[core]["y"]
            .astype(np.float32)
            .reshape(128, M_CHUNKS, TOK_PER)
            .transpose(1, 0, 2)  # [g, p, t]
        )
        y_log = y_dev[[0, 4, 1, 5, 2, 6, 3, 7]].reshape(OUT_PER, TOK_PER)
        y[t * TOK_PER : (t + 1) * TOK_PER, h * OUT_PER : (h + 1) * OUT_PER] = y_log.T
    return (y + bias_f[None, :]).reshape(BATCH, SEQ, OUT_F)
